# revision 1
# baseline (speedup 1.0000x reference)
"""Trainium2 Bass kernel for nn_EEGMI_RWKV_ResNet_Model.

Sharding: data-parallel over batch. B=32 -> 4 batches on each of 8 cores.
All parameters are replicated (host-preprocessed: BN folded into conv
weights, channel order permuted to o' = j*64 + c, channels padded 320->384,
weights pre-transposed into lhsT layouts, bf16 casts done on host).

Device layout conventions (per core, BL=4 local batches):
  - conv stages: channel-on-partition "H-part" tiles (128, 2052) bf16,
    data in cols [2, 2050), zero padding columns for the shifted conv taps.
  - rwkv stages: (128, 2048) bf16 tiles, H on partitions, T on free dim.
  - The wkv scan runs on the vector engine via tensor_tensor_scan
    (alpha_t = 0.9*alpha_{t-1} + s_t), then wkv_t = alpha_t + 0.1*alpha_{t-1}.
  - LayerNorm over H (partition axis): sums via ones-vector matmuls,
    per-t scalars broadcast back over partitions via K=1 matmuls.

kernel(**inputs) is self-contained: numpy preprocessing + bass build +
run_bass_kernel_spmd on cores 0..7 + gather.
"""
import os
import numpy as np
import ml_dtypes

import concourse.bass as bass
import concourse.bacc as bacc
import concourse.tile as tile
from concourse import mybir
from concourse.bass_utils import run_bass_kernel_spmd

EPS = 1e-5
B, T, C = 32, 2048, 64
NB, C5, H, L, NBLK, NCLS = 5, 320, 128, 3, 2, 4
CP = 384
NCORE = 8
BL = B // NCORE
NCH = 4
CH = 512
TP = T + 4      # padded width for band conv input
TF = T + 4      # feat tiles width (data cols 2..2050); >= T+2 needed
NT16 = 16       # 128-col chunks for transpose

PERM = np.array([(o % 64) * 5 + (o // 64) for o in range(C5)], dtype=np.int64)

F32 = mybir.dt.float32
F32R = mybir.dt.float32r
BF16 = mybir.dt.bfloat16
AF = mybir.ActivationFunctionType
ALU = mybir.AluOpType
bf16np = ml_dtypes.bfloat16


# ---------------------------------------------------------------------------
# host-side weight preprocessing (numpy only)
# ---------------------------------------------------------------------------

def _prep_weights(inp):
    f32 = np.float32
    out = {}

    bw = np.asarray(inp['band_w'], f32)[PERM, 0, :]
    bb = np.asarray(inp['band_b'], f32)[PERM]
    bw_pad = np.zeros((CP, 5), f32); bw_pad[:C5] = bw
    bb_pad = np.zeros((CP,), f32); bb_pad[:C5] = bb
    band_coef = bw_pad.reshape(3, 128, 5)

    bw_raw = np.asarray(inp['band_w'], f32)[:, 0, :].reshape(C, NB, 5)
    denom = f32(1.0 / (NB * T))
    A = bw_raw.sum(axis=(1, 2)) * denom
    E0 = -(bw_raw[:, :, 3] + bw_raw[:, :, 4]).sum(1) * denom
    E1 = -(bw_raw[:, :, 4]).sum(1) * denom
    E2 = -(bw_raw[:, :, 0]).sum(1) * denom
    E3 = -(bw_raw[:, :, 0] + bw_raw[:, :, 1]).sum(1) * denom
    Bb = np.asarray(inp['band_b'], f32).reshape(C, NB).mean(1)

    attn_rhs = np.zeros((65, 64), f32)
    attn_rhs[:64] = np.asarray(inp['attn_w'], f32).T
    attn_rhs[64] = np.asarray(inp['attn_b'], f32)
    out['attn_rhs'] = attn_rhs

    res_lhsT = np.zeros((4, 3, 3, 3, 128, 128), f32)
    res_bias = np.zeros((4, CP), f32)
    ci = 0
    for blk in range(NBLK):
        for lyr in range(2):
            W = np.asarray(inp['res_w'], np.float32)[blk, lyr]
            g = np.asarray(inp['res_bn_g'], f32)[blk, lyr]
            b = np.asarray(inp['res_bn_b'], f32)[blk, lyr]
            m = np.asarray(inp['res_bn_m'], f32)[blk, lyr]
            v = np.asarray(inp['res_bn_v'], f32)[blk, lyr]
            inv = g / np.sqrt(v + EPS)
            Wf = W * inv[:, None, None]
            bf = b - m * inv
            Wp = Wf[PERM][:, PERM]
            Wpad = np.zeros((CP, CP, 3), f32); Wpad[:C5, :C5] = Wp
            bpad = np.zeros((CP,), f32); bpad[:C5] = bf[PERM]
            res_bias[ci] = bpad
            WT = Wpad.transpose(1, 0, 2)
            for k in range(3):
                for q in range(3):
                    for mm in range(3):
                        res_lhsT[ci, k, q, mm] = \
                            WT[q*128:(q+1)*128, mm*128:(mm+1)*128, k]
            ci += 1
    out['res_lhsT'] = np.ascontiguousarray(
        res_lhsT.transpose(4, 0, 1, 2, 3, 5)).astype(bf16np)

    pw = np.asarray(inp['proj_w'], f32)[:, PERM]
    pw_pad = np.zeros((H, CP), f32); pw_pad[:, :C5] = pw
    out['proj_lhsT'] = np.ascontiguousarray(
        pw_pad.T.reshape(3, 128, H).transpose(1, 0, 2)).astype(bf16np)

    rwkv_lhsT = np.zeros((L, 4, H, H), f32)
    for l in range(L):
        rwkv_lhsT[l, 0] = np.asarray(inp['wk'], f32)[l].T
        rwkv_lhsT[l, 1] = np.asarray(inp['wv'], f32)[l].T
        rwkv_lhsT[l, 2] = np.asarray(inp['wr'], f32)[l].T
        rwkv_lhsT[l, 3] = np.asarray(inp['wo'], f32)[l].T
    out['rwkv_lhsT'] = np.ascontiguousarray(
        rwkv_lhsT.transpose(2, 0, 1, 3)).astype(bf16np)

    w1 = np.asarray(inp['cls_w1'], f32)
    out['cls1_lhsT'] = np.ascontiguousarray(w1.T.reshape(H, 2, 128))
    w2 = np.asarray(inp['cls_w2'], f32)
    out['cls2_lhsT'] = np.ascontiguousarray(
        w2.T.reshape(2, 128, NCLS).transpose(1, 0, 2))

    cols = {}
    def vec(name, v):
        cols[name] = np.asarray(v, f32)
    def pad128(v):
        o = np.zeros(128, f32); o[:len(v)] = v; return o

    for i in range(3):
        for k in range(5):
            vec(f'band_c{i}_{k}', band_coef[i, :, k])
    vec('A', pad128(A)); vec('E0', pad128(E0)); vec('E1', pad128(E1))
    vec('E2', pad128(E2)); vec('E3', pad128(E3)); vec('Bb', pad128(Bb))
    for c4 in range(4):
        for mm in range(3):
            vec(f'res_b{c4}_{mm}', res_bias[c4, mm*128:(mm+1)*128])
    vec('proj_b', np.asarray(inp['proj_b'], f32))
    for l in range(L):
        for w, nm in enumerate(['tmk', 'tmv', 'tmr']):
            tm = np.asarray(inp[nm], f32)[l]
            vec(f'tm{l}_{w}', tm)
            vec(f'tm1_{l}_{w}', (1.0 - tm) / T)
        vec(f'ln1g_{l}', np.asarray(inp['ln1g'], f32)[l])
        vec(f'ln1b_{l}', np.asarray(inp['ln1b'], f32)[l])
        vec(f'ln2g_{l}', np.asarray(inp['ln2g'], f32)[l])
        vec(f'ln2b_{l}', np.asarray(inp['ln2b'], f32)[l])
    for i in range(3):
        vec(f'band_bias_{i}', bb_pad.reshape(3, 128)[i])
    vec('cls_b1a', np.asarray(inp['cls_b1'], f32)[:128])
    vec('cls_b1b', np.asarray(inp['cls_b1'], f32)[128:])
    vec('cls_b2', pad128(np.asarray(inp['cls_b2'], f32)))
    vec('eps', np.full(128, EPS, f32))

    names = list(cols.keys())
    out['cvec'] = np.ascontiguousarray(np.stack([cols[n] for n in names], 1))
    out['cvec_idx'] = {n: i for i, n in enumerate(names)}
    out['identity'] = np.eye(128, dtype=f32)
    return out


# ---------------------------------------------------------------------------
# bass kernel builder
# ---------------------------------------------------------------------------

def _build_nc(nv, dbg_keys=()):
    """nv = number of cvec columns."""
    nc = bacc.Bacc(None, target_bir_lowering=False)

    d_x = nc.dram_tensor('x', [BL, 128, TP], BF16, kind='ExternalInput')
    d_cvec = nc.dram_tensor('cvec', [128, nv], F32, kind='ExternalInput')
    d_attn = nc.dram_tensor('attn_rhs', [65, 64], F32R, kind='ExternalInput')
    d_res = nc.dram_tensor('res_lhsT', [128, 4, 3, 3, 3, 128], BF16,
                           kind='ExternalInput')
    d_proj = nc.dram_tensor('proj_lhsT', [128, 3, H], BF16, kind='ExternalInput')
    d_rwkv = nc.dram_tensor('rwkv_lhsT', [128, L, 4, H], BF16, kind='ExternalInput')
    d_cls1 = nc.dram_tensor('cls1_lhsT', [128, 2, 128], F32R, kind='ExternalInput')
    d_cls2 = nc.dram_tensor('cls2_lhsT', [128, 2, NCLS], F32R, kind='ExternalInput')
    d_out = nc.dram_tensor('out', [NCLS, BL], F32, kind='ExternalOutput')

    with tile.TileContext(nc) as tc:
        _emit(nc, tc, d_x, d_cvec, d_attn, d_res, d_proj, d_rwkv,
              d_cls1, d_cls2, d_out, nv, dbg_keys)
    nc.finalize()
    return nc


def _emit(nc, tc, d_x, d_cvec, d_attn, d_res, d_proj, d_rwkv,
          d_cls1, d_cls2, d_out, nv, dbg_keys=()):
    from contextlib import ExitStack

    def cap(key, ap):
        if key in dbg_keys:
            dt = nc.dram_tensor(f'dbg_{key}', list(ap.shape),
                                ap.dtype, kind='ExternalOutput')
            nc.gpsimd.dma_start(out=dt[...], in_=ap)

    ctx = ExitStack()
    with ctx:
        consts = ctx.enter_context(tc.tile_pool(name='consts', bufs=1))
        big = ctx.enter_context(tc.tile_pool(name='big', bufs=28))
        stats = ctx.enter_context(tc.tile_pool(name='stats', bufs=1))
        small = ctx.enter_context(tc.tile_pool(name='small', bufs=1))
        xload = ctx.enter_context(tc.tile_pool(name='xload', bufs=4))
        psum = ctx.enter_context(tc.tile_pool(name='psum', bufs=8, space='PSUM'))
        psum_s = psum

        def bigt(name):
            return big.tile([128, TF], BF16, tag='big', name=name)

        # ---------------- constants -----------------
        cvec = consts.tile([128, nv], F32)
        nc.gpsimd.dma_start(out=cvec, in_=d_cvec[:, :])
        CV = {}

        def colap(name):
            return cvec[:, CV[name]:CV[name]+1]

        idx = 0
        def reg(name):
            nonlocal idx
            CV[name] = idx; idx += 1
        for i in range(3):
            for k in range(5):
                reg(f'band_c{i}_{k}')
        for n in ['A', 'E0', 'E1', 'E2', 'E3', 'Bb']:
            reg(n)
        for c4 in range(4):
            for mm in range(3):
                reg(f'res_b{c4}_{mm}')
        reg('proj_b')
        for l in range(L):
            for w in range(3):
                reg(f'tm{l}_{w}')
                reg(f'tm1_{l}_{w}')
            for n in [f'ln1g_{l}', f'ln1b_{l}', f'ln2g_{l}', f'ln2b_{l}']:
                reg(n)
        for i in range(3):
            reg(f'band_bias_{i}')
        for n in ['cls_b1a', 'cls_b1b', 'cls_b2', 'eps']:
            reg(n)
        assert idx == nv, (idx, nv)

        ones_l = consts.tile([128, 1], BF16)
        nc.vector.memset(ones_l, 1.0)
        decay = consts.tile([128, T], F32)
        nc.vector.memset(decay, 0.9)
        # f32r tiles cannot be memset directly (invalid ISA); synthesize via
        # ACT: out = Copy(in*0 + 1)
        ones_lf = consts.tile([128, 128], F32R)
        nc.scalar.activation(out=ones_lf, in_=decay[:, 0:128], func=AF.Copy,
                             bias=1.0, scale=0.0)

        attn_rhs = consts.tile([65, 64], F32R)
        nc.gpsimd.dma_start(out=attn_rhs, in_=d_attn[:, :])

        w_res = consts.tile([128, 4, 3, 3, 3, 128], BF16)
        nc.gpsimd.dma_start(out=w_res, in_=d_res[...])
        w_proj = consts.tile([128, 3, H], BF16)
        nc.gpsimd.dma_start(out=w_proj, in_=d_proj[...])
        w_rwkv = consts.tile([128, L, 4, H], BF16)
        nc.gpsimd.dma_start(out=w_rwkv, in_=d_rwkv[...])
        w_cls1 = consts.tile([128, 2, 128], F32R)
        nc.gpsimd.dma_start(out=w_cls1, in_=d_cls1[...])
        w_cls2 = consts.tile([128, 2, NCLS], F32R)
        nc.gpsimd.dma_start(out=w_cls2, in_=d_cls2[...])

        # ---------------- stage 1: load x, band conv ------------
        # x arrives host-packed as (BL, 128, TP) bf16: channels on the
        # partition dim (duplicated [c; c]) with 2 zero pad cols per side,
        # i.e. the exact band-conv input layout. One contiguous DMA each.
        xdup = [bigt(f'xdup{b}') for b in range(BL)]
        for b in range(BL):
            nc.sync.dma_start(out=xdup[b][:, 0:TP], in_=d_x[b, :, :])
        S_b = small.tile([64, BL], F32)
        for b in range(BL):
            nc.vector.tensor_reduce(
                out=S_b[:, b:b+1], in_=xdup[b][0:64, 2:2+T],
                axis=mybir.AxisListType.X, op=ALU.add)
        cap('xdup0', xdup[0][:, :])
        cap('S_b', S_b[:, :])

        F = [[bigt(f'F{b}_{i}') for i in range(3)] for b in range(BL)]
        O = [[bigt(f'O{b}_{i}') for i in range(3)] for b in range(BL)]
        for b in range(BL):
            for i in range(3):
                nc.vector.memset(F[b][i], 0.0)
                nc.vector.memset(O[b][i], 0.0)

        # band conv: F[b][i][:, 2:2+T] = sum_k coef_ik * xdup[b][:, k:k+T]
        for b in range(BL):
            for i in range(3):
                rows = 128 if i < 2 else 64
                dst = F[b][i][0:rows, 2:2+T]
                for k in range(5):
                    src = xdup[b][0:rows, k:k+T]
                    cf = colap(f'band_c{i}_{k}')[0:rows]
                    if k == 0:
                        nc.vector.tensor_scalar(
                            out=dst, in0=src, scalar1=cf, scalar2=None,
                            op0=ALU.mult)
                    else:
                        nc.vector.scalar_tensor_tensor(
                            out=dst, in0=src, scalar=cf, in1=dst,
                            op0=ALU.mult, op1=ALU.add)

        for i in range(3):
            cap(f'F0{i}_band', F[0][i][:, :])
        # pooled (64, BL)
        pooledT = small.tile([65, BL], F32R)
        nc.scalar.activation(out=pooledT[64:65, :], in_=S_b[0:1, 0:BL],
                             func=AF.Copy, bias=1.0, scale=0.0)
        for b in range(BL):
            p = pooledT[0:64, b:b+1]
            nc.vector.tensor_scalar(
                out=p, in0=S_b[:, b:b+1], scalar1=colap('A')[0:64],
                scalar2=colap('Bb')[0:64], op0=ALU.mult, op1=ALU.add)
            for name, cc in [('E0', 2), ('E1', 3), ('E2', T), ('E3', T+1)]:
                nc.vector.scalar_tensor_tensor(
                    out=p, in0=xdup[b][0:64, cc:cc+1],
                    scalar=colap(name)[0:64], in1=p,
                    op0=ALU.mult, op1=ALU.add)

        # attention: softmax over the 64 channels, computed in transposed
        # form (64, BL) so no PE transpose is needed. Logits are O(1) so the
        # max-subtraction can be dropped.
        att_ps = psum_s.tile([64, BL], F32, tag='mm512', name='att_ps')
        nc.tensor.matmul(att_ps, attn_rhs, pooledT)
        attE = small.tile([64, BL], F32R)
        nc.scalar.activation(out=attE, in_=att_ps, func=AF.Exp)
        sum_ps = psum_s.tile([1, BL], F32, tag='mm512', name='sum_ps')
        nc.tensor.matmul(sum_ps, ones_lf[0:64, 0:1], attE)
        arec = small.tile([1, BL], F32R)
        with nc.allow_low_precision(reason='softmax denom in fp32r is fine'):
            nc.vector.reciprocal(out=arec, in_=sum_ps)
        bc_ps = psum_s.tile([64, BL], F32, tag='mm512', name='bc_ps')
        nc.tensor.matmul(bc_ps, ones_lf[0:1, 0:64], arec, tile_position=(0, 0))
        attT = small.tile([64, BL], F32)
        nc.vector.tensor_tensor(out=attT, in0=attE, in1=bc_ps, op=ALU.mult)
        cap('pooledT', pooledT[:, :])
        cap('attT', attT[:, :])
        avec = [[small.tile([128, 1], F32, tag='avec', bufs=12,
                            name=f'avec{b}_{i}') for i in range(3)]
                for b in range(BL)]
        bxa = [[small.tile([128, 1], F32, tag='bxa', bufs=12,
                           name=f'bxa{b}_{i}') for i in range(3)]
               for b in range(BL)]
        for b in range(BL):
            for i in range(3):
                nc.vector.memset(avec[b][i], 0.0)
                nc.gpsimd.dma_start(out=avec[b][i][0:64, :], in_=attT[:, b:b+1])
                if i < 2:
                    nc.gpsimd.dma_start(out=avec[b][i][64:128, :],
                                      in_=attT[:, b:b+1])
                nc.vector.tensor_tensor(
                    out=bxa[b][i], in0=avec[b][i],
                    in1=colap(f'band_bias_{i}'), op=ALU.mult)
        for b in range(BL):
            for i in range(3):
                nc.vector.tensor_scalar(
                    out=F[b][i][:, 2:2+T], in0=F[b][i][:, 2:2+T],
                    scalar1=avec[b][i], scalar2=bxa[b][i],
                    op0=ALU.mult, op1=ALU.add)

        # ---------------- stage 2: resnet ---------------------------------
        def conv(c4, IN, OUT, residual):
            groups = [(b, n) for b in range(BL) for n in range(NCH)]
            for m in range(3):
                for gi in range(0, 16, 8):
                    gs = groups[gi:gi+8]
                    pts = [psum.tile([128, CH], F32, tag='mm512',
                                     name=f'cvp{c4}_{m}_{gi}_{g}')
                           for g in range(len(gs))]
                    first = True
                    for k in range(3):
                        for q in range(3):
                            lhsT = w_res[:, c4, k, q, m, :]
                            for (b, n), pt in zip(gs, pts):
                                rhs = IN[b][q][:, 1 + CH*n + k: 1 + CH*n + k + CH]
                                nc.tensor.matmul(
                                    pt, lhsT, rhs, start=first,
                                    stop=(k == 2 and q == 2))
                            first = False
                    bias = colap(f'res_b{c4}_{m}')
                    for (b, n), pt in zip(gs, pts):
                        dst = OUT[b][m][:, 2 + CH*n: 2 + CH*(n+1)]
                        if not residual:
                            nc.scalar.activation(out=dst, in_=pt, func=AF.Relu,
                                                 bias=bias, scale=1.0)
                        else:
                            tmp = xload.tile([128, CH], BF16, tag='cvt',
                                             name=f'cvt{c4}_{m}_{gi}_{b}_{n}')
                            nc.vector.scalar_tensor_tensor(
                                out=tmp, in0=pt, scalar=bias, in1=dst,
                                op0=ALU.add, op1=ALU.add)
                            nc.vector.tensor_scalar(
                                out=dst, in0=tmp, scalar1=0.0, scalar2=None,
                                op0=ALU.max)

        cap('F00_scaled', F[0][0][:, :])
        if 'noconv' not in os.environ.get('KABL', ''):
            conv(0, F, O, residual=False)
            cap('O00_c1', O[0][0][:, :])
            conv(1, O, F, residual=True)
            cap('F00_b1', F[0][0][:, :])
            conv(2, F, O, residual=False)
            conv(3, O, F, residual=True)
        cap('F00_res', F[0][0][:, :])

        # ---------------- stage 3: proj -----------------------------------
        h = [bigt(f'h{b}') for b in range(BL)]
        sums = [small.tile([128, 1], F32, tag='hsum', bufs=8,
                           name=f'hsum{b}') for b in range(BL)]
        for b in range(BL):
            for n in range(NCH):
                pt = psum.tile([128, CH], F32, tag='mm512', name=f'pjp{b}_{n}')
                for q in range(3):
                    nc.tensor.matmul(pt, w_proj[:, q, :],
                                     F[b][q][:, 2 + CH*n: 2 + CH*(n+1)],
                                     start=(q == 0), stop=(q == 2))
                nc.scalar.activation(out=h[b][:, CH*n:CH*(n+1)], in_=pt,
                                     func=AF.Identity, bias=colap('proj_b'),
                                     scale=1.0)
            nc.vector.tensor_reduce(out=sums[b], in_=h[b][:, 0:T],
                                    axis=mybir.AxisListType.X, op=ALU.add)
        cap('h0', h[0][:, 0:T])

        # ---------------- stage 4: rwkv layers -----------------------------
        nlayers = 0 if 'norwkv' in os.environ.get('KABL', '') else L
        for l in range(nlayers):
            h, sums = _rwkv_layer(nc, big, bigt, small, xload, psum, psum_s,
                                  stats, colap, w_rwkv, ones_l, ones_lf,
                                  decay, h, sums, l, cap)
            cap(f'hn{l}_0', h[0][:, 0:T])

        # ---------------- stage 5: head ------------------------------------
        pooledHf = small.tile([128, BL], F32R)
        for b in range(BL):
            nc.vector.tensor_scalar(out=pooledHf[:, b:b+1], in0=sums[b],
                                    scalar1=1.0 / T, scalar2=None, op0=ALU.mult)
        hidT = small.tile([128, 2, BL], F32R)
        for mt in range(2):
            pt = psum_s.tile([128, BL], F32, tag='mm512', name=f'clsp{mt}')
            nc.tensor.matmul(pt, w_cls1[:, mt, :],
                             pooledHf)
            nc.scalar.activation(out=hidT[:, mt, :], in_=pt, func=AF.Relu,
                                 bias=colap('cls_b1a' if mt == 0 else 'cls_b1b'),
                                 scale=1.0)
        out_ps = psum_s.tile([NCLS, BL], F32, tag='mm512', name='out_ps')
        for kt in range(2):
            nc.tensor.matmul(out_ps, w_cls2[:, kt, :],
                             hidT[:, kt, :],
                             start=(kt == 0), stop=(kt == 1))
        cap('pooledHf', pooledHf[:, :])
        cap('hidT', hidT[:, :, :])
        out_sb = small.tile([NCLS, BL], F32)
        nc.scalar.activation(out=out_sb, in_=out_ps, func=AF.Identity,
                             bias=colap('cls_b2')[0:NCLS], scale=1.0)
        nc.gpsimd.dma_start(out=d_out[:, :], in_=out_sb)


def _rwkv_layer(nc, big, bigt, small, xload, psum, psum_s, stats, colap,
                w_rwkv, ones_l, ones_lf, decay, h, sums, l, cap=lambda *a: None):
    # xk/xv/xr
    xs = [[bigt(f'xs{l}_{b}_{w}') for w in range(3)] for b in range(BL)]
    for b in range(BL):
        for w in range(3):
            tmv1 = small.tile([128, 1], F32, tag='tmv1', bufs=4,
                              name=f'tmv1_{l}_{b}_{w}')
            nc.vector.tensor_tensor(out=tmv1, in0=sums[b],
                                    in1=colap(f'tm1_{l}_{w}'), op=ALU.mult)
            nc.vector.tensor_scalar(
                out=xs[b][w][:, 0:T], in0=h[b][:, 0:T],
                scalar1=colap(f'tm{l}_{w}'),
                scalar2=tmv1, op0=ALU.mult, op1=ALU.add)
    sk = [bigt(f'sk{l}_{b}') for b in range(BL)]
    vv = [bigt(f'vv{l}_{b}') for b in range(BL)]
    rr = [bigt(f'rr{l}_{b}') for b in range(BL)]
    for b in range(BL):
        for w, (dst, fn) in enumerate([(sk[b], AF.Sigmoid), (vv[b], AF.Relu),
                                       (rr[b], AF.Sigmoid)]):
            for n in range(NCH):
                pt = psum.tile([128, CH], F32, tag='mm512',
                               name=f'kvr{l}_{b}_{w}_{n}')
                nc.tensor.matmul(pt, w_rwkv[:, l, w, :],
                                 xs[b][w][:, CH*n:CH*(n+1)])
                nc.scalar.activation(out=dst[:, CH*n:CH*(n+1)], in_=pt, func=fn)
    ss = [bigt(f'ss{l}_{b}') for b in range(BL)]
    alpha = [bigt(f'alpha{l}_{b}') for b in range(BL)]
    rwkv = [bigt(f'rwkv{l}_{b}') for b in range(BL)]
    for b in range(BL):
        nc.vector.scalar_tensor_tensor(
            out=ss[b][:, 0:T], in0=sk[b][:, 0:T], scalar=0.5,
            in1=vv[b][:, 0:T], op0=ALU.max, op1=ALU.mult)
        nc.vector.memset(alpha[b][:, 0:1], 0.0)
        import os as _os
        if 'noscan' in _os.environ.get('KABL', ''):
            nc.vector.tensor_copy(out=alpha[b][:, 1:T+1], in_=ss[b][:, 0:T])
        else:
            nc.vector.tensor_tensor_scan(
                out=alpha[b][:, 1:T+1], data0=decay, data1=ss[b][:, 0:T],
                initial=0.0, op0=ALU.mult, op1=ALU.add)
        nc.vector.scalar_tensor_tensor(
            out=ss[b][:, 0:T], in0=alpha[b][:, 0:T], scalar=0.1,
            in1=alpha[b][:, 1:T+1], op0=ALU.mult, op1=ALU.add)
        nc.vector.tensor_tensor(out=rwkv[b][:, 0:T], in0=rr[b][:, 0:T],
                                in1=ss[b][:, 0:T], op=ALU.mult)
    if l == 0:
        cap('xs00', xs[0][0][:, 0:T])
        cap('sk00', sk[0][:, 0:T])
        cap('vv00', vv[0][:, 0:T])
        cap('rr00', rr[0][:, 0:T])
        cap('alpha00', alpha[0][:, 0:T+1])
        cap('rwkv00', rwkv[0][:, 0:T])
    y = [bigt(f'y{l}_{b}') for b in range(BL)]
    for b in range(BL):
        for n in range(NCH):
            pt = psum.tile([128, CH], F32, tag='mm512', name=f'op{l}_{b}_{n}')
            nc.tensor.matmul(pt, w_rwkv[:, l, 3, :], rwkv[b][:, CH*n:CH*(n+1)])
            nc.vector.tensor_tensor(out=y[b][:, CH*n:CH*(n+1)], in0=pt,
                                    in1=h[b][:, CH*n:CH*(n+1)], op=ALU.add)

    if l == 0:
        cap('y00', y[0][:, 0:T])
    yn = [bigt(f'yn{l}_{b}') for b in range(BL)]
    ffp = [bigt(f'ffp{l}_{b}') for b in range(BL)]
    hn = [bigt(f'hn{l}_{b}') for b in range(BL)]
    nsums = [small.tile([128, 1], F32, tag='hsum', bufs=8,
                        name=f'nsums{l}_{b}') for b in range(BL)]
    _ln(nc, big, bigt, small, xload, psum, stats, colap, ones_l, ones_lf,
        y, yn, f'ln1g_{l}', f'ln1b_{l}', tagp=f'l{l}a')
    _ln(nc, big, bigt, small, xload, psum, stats, colap, ones_l, ones_lf,
        yn, ffp, f'ln2g_{l}', f'ln2b_{l}', tagp=f'l{l}b')
    if l == 0:
        cap('yn00', yn[0][:, 0:T])
        cap('ffp00', ffp[0][:, 0:T])
    for b in range(BL):
        nc.vector.scalar_tensor_tensor(
            out=hn[b][:, 0:T], in0=ffp[b][:, 0:T], scalar=0.0,
            in1=yn[b][:, 0:T], op0=ALU.max, op1=ALU.add, accum_out=nsums[b])
    return hn, nsums


def _ln(nc, big, bigt, small, xload, psum, stats, colap, ones_l, ones_lf,
        y, out, gname, bname, tagp):
    """LayerNorm over the partition axis for each (batch, t) column.
    Stats rows live at partition 32*b of (128, T) f32 tiles."""
    stat_y = stats.tile([128, T], F32R, tag='stat_y', name=f'sty_{tagp}')
    stat_q = stats.tile([128, T], F32R, tag='stat_q', name=f'stq_{tagp}')
    stat_v = stats.tile([128, T], F32, tag='stat_v', name=f'stv_{tagp}')
    ysq = [bigt(f'ysq{tagp}_{b}') for b in range(BL)]
    for b in range(BL):
        nc.scalar.activation(out=ysq[b][:, 0:T], in_=y[b][:, 0:T],
                             func=AF.Square)
    for n in range(NCH):
        p1 = psum.tile([128, CH], F32, tag='mm512', name=f'st1_{tagp}_{n}')
        p2 = psum.tile([128, CH], F32, tag='mm512', name=f'st2_{tagp}_{n}')
        for b in range(BL):
            nc.tensor.matmul(p1[32*b:32*b+1, :], ones_l,
                             y[b][:, CH*n:CH*(n+1)], tile_position=(0, 32*b))
            nc.tensor.matmul(p2[32*b:32*b+1, :], ones_l,
                             ysq[b][:, CH*n:CH*(n+1)], tile_position=(0, 32*b))
        nc.scalar.activation(out=stat_y[:, CH*n:CH*(n+1)], in_=p1, func=AF.Copy,
                             scale=1.0 / H)
        nc.scalar.activation(out=stat_q[:, CH*n:CH*(n+1)], in_=p2, func=AF.Copy,
                             scale=1.0 / H)
    sp = lambda t: t  # full-range ops; only rows 32*b are meaningful
    # var = e2 - mu^2 (into stat_q); sd = sqrt(var+eps) (stat_v);
    # inv = 1/sd (stat_q); negq = -mu*inv (stat_y)
    nc.vector.tensor_tensor(out=sp(stat_v), in0=sp(stat_y), in1=sp(stat_y),
                            op=ALU.mult)
    nc.vector.tensor_tensor(out=sp(stat_q), in0=sp(stat_q), in1=sp(stat_v),
                            op=ALU.subtract)
    nc.scalar.activation(out=sp(stat_v), in_=sp(stat_q), func=AF.Sqrt,
                         bias=colap('eps'), scale=1.0)
    with nc.allow_low_precision(reason='fp32r LN inv is plenty (FP22)'):
        nc.vector.reciprocal(out=sp(stat_q), in_=sp(stat_v))
    nc.vector.scalar_tensor_tensor(out=sp(stat_y), in0=sp(stat_y), scalar=-1.0,
                                   in1=sp(stat_q), op0=ALU.mult, op1=ALU.mult)
    inv, negq = stat_q, stat_y
    gv = colap(gname); bv = colap(bname)
    for b in range(BL):
        pb = bigt(f'bcP{tagp}_{b}')
        qb = bigt(f'bcQ{tagp}_{b}')
        for n in range(NCH):
            bp = psum.tile([128, CH], F32, tag='mm512', name=f'bp_{tagp}_{b}_{n}')
            bq = psum.tile([128, CH], F32, tag='mm512', name=f'bq_{tagp}_{b}_{n}')
            nc.tensor.matmul(bp, ones_lf[32*b:32*b+1, :],
                             inv[32*b:32*b+1, CH*n:CH*(n+1)],
                             tile_position=(32*b, 0))
            nc.tensor.matmul(bq, ones_lf[32*b:32*b+1, :],
                             negq[32*b:32*b+1, CH*n:CH*(n+1)],
                             tile_position=(32*b, 0))
            nc.scalar.activation(out=pb[:, CH*n:CH*(n+1)], in_=bp,
                                 func=AF.Identity, bias=0.0, scale=gv)
            nc.scalar.activation(out=qb[:, CH*n:CH*(n+1)], in_=bq,
                                 func=AF.Identity, bias=bv, scale=gv)
        t1 = bigt(f'lnt{tagp}_{b}')
        nc.vector.tensor_tensor(out=t1[:, 0:T], in0=y[b][:, 0:T],
                                in1=pb[:, 0:T], op=ALU.mult)
        nc.vector.tensor_tensor(out=out[b][:, 0:T], in0=t1[:, 0:T],
                                in1=qb[:, 0:T], op=ALU.add)


# ---------------------------------------------------------------------------
# entry point
# ---------------------------------------------------------------------------

_CACHE = {}


def kernel(**inputs):
    prep = _prep_weights(inputs)
    nv = prep['cvec'].shape[1]
    if 'nc' not in _CACHE:
        _CACHE['nc'] = _build_nc(nv)
    nc = _CACHE['nc']

    x = np.asarray(inputs['x'], np.float32).astype(bf16np)
    xc = x.reshape(NCORE, BL, T, C).transpose(0, 1, 3, 2)   # (core, b, c, t)
    xs = np.zeros((NCORE, BL, 128, TP), dtype=bf16np)
    xs[:, :, 0:64, 2:2+T] = xc
    xs[:, :, 64:128, 2:2+T] = xc
    shared = {
        'cvec': prep['cvec'],
        'attn_rhs': prep['attn_rhs'], 'res_lhsT': prep['res_lhsT'],
        'proj_lhsT': prep['proj_lhsT'], 'rwkv_lhsT': prep['rwkv_lhsT'],
        'cls1_lhsT': prep['cls1_lhsT'], 'cls2_lhsT': prep['cls2_lhsT'],
    }
    in_maps = [dict(shared, x=np.ascontiguousarray(xs[c]))
               for c in range(NCORE)]
    _CACHE['in_maps'] = in_maps
    res = run_bass_kernel_spmd(nc, in_maps, core_ids=list(range(NCORE)))
    outs = [res.results[c]['out'] for c in range(NCORE)]   # (NCLS, BL) each
    logits = np.concatenate([o.T for o in outs], axis=0)   # (B, NCLS)
    return logits.astype(np.float32)


def profile_exec_ns():
    """Re-run with NTFF tracing; returns max per-core exec time in ns."""
    if 'nc' not in _CACHE or 'in_maps' not in _CACHE:
        return None
    try:
        res = run_bass_kernel_spmd(_CACHE['nc'], _CACHE['in_maps'],
                                   core_ids=list(range(NCORE)), trace=True)
    except Exception as e:
        print("profile unavailable:", e)
        return None
    if res.instructions_and_trace is not None:
        print("trace:", res.instructions_and_trace[1])
    return res.exec_time_ns


def bench_exec(n=8):
    """Steady-state timing of the compiled SPMD executable (device-resident
    inputs, jit built once). Returns (min_s, avg_s) per call."""
    import time
    import jax
    from jax.sharding import Mesh, PartitionSpec
    from jax.experimental.shard_map import shard_map
    from concourse import bass2jax as b2j
    from concourse import mybir

    nc = _CACHE['nc']; in_maps = _CACHE['in_maps']
    b2j.install_neuronx_cc_hook()
    partition_name = nc.partition_id_tensor.name if nc.partition_id_tensor else None
    in_names, out_names, out_avals, zero_outs = [], [], [], []
    for alloc in nc.m.functions[0].allocations:
        if not isinstance(alloc, mybir.MemoryLocationSet):
            continue
        name = alloc.memorylocations[0].name
        if alloc.kind == 'ExternalInput':
            if name != partition_name:
                in_names.append(name)
        elif alloc.kind == 'ExternalOutput':
            sh = tuple(alloc.tensor_shape)
            dt = mybir.dt.np(alloc.dtype)
            out_avals.append(jax.core.ShapedArray(sh, dt))
            out_names.append(name)
            zero_outs.append(np.zeros(sh, dt))
    n_params = len(in_names)
    n_outs = len(out_avals)
    all_in_names = list(in_names) + list(out_names)
    if partition_name is not None:
        all_in_names.append(partition_name)

    def _body(*args):
        operands = list(args)
        if partition_name is not None:
            operands.append(b2j.partition_id_tensor())
        outs = b2j._bass_exec_p.bind(
            *operands, out_avals=tuple(out_avals), in_names=tuple(all_in_names),
            out_names=tuple(out_names), lowering_input_output_aliases=(),
            sim_require_finite=True, sim_require_nnan=True, nc=nc)
        return tuple(outs)

    devices = jax.devices()[:NCORE]
    mesh = Mesh(np.asarray(devices), ('core',))
    in_specs = (PartitionSpec('core'),) * (n_params + n_outs)
    out_specs = (PartitionSpec('core'),) * len(out_names)
    sharded = jax.jit(shard_map(_body, mesh=mesh, in_specs=in_specs,
                                out_specs=out_specs, check_rep=False),
                      keep_unused=True)
    concat_in = [np.concatenate([np.asarray(in_maps[c][nm])
                                 for c in range(NCORE)], axis=0)
                 for nm in in_names]
    concat_zeros = [np.zeros((NCORE * z.shape[0], *z.shape[1:]), z.dtype)
                    for z in zero_outs]
    args = [jax.device_put(a) for a in concat_in + concat_zeros]
    r = sharded(*args); jax.block_until_ready(r)   # warmup/compile
    def run_n(k):
        t0 = time.perf_counter()
        rs = [sharded(*args) for _ in range(k)]
        jax.block_until_ready(rs)
        return time.perf_counter() - t0
    run_n(2)
    t1 = min(run_n(1) for _ in range(3))
    tn = min(run_n(n) for _ in range(3))
    slope = (tn - t1) / (n - 1)
    return t1, slope



# revision 4
# speedup vs baseline: 8.4798x; 8.4798x over previous
"""Trainium2 Bass kernel for nn_EEGMI_RWKV_ResNet_Model.

Sharding: data-parallel over batch. B=32 -> 4 batches on each of 8 cores.
All parameters are replicated (host-preprocessed: BN folded into conv
weights, channel order permuted to o' = j*64 + c, channels padded 320->384,
weights pre-transposed into lhsT layouts, bf16 casts done on host).

Device layout conventions (per core, BL=4 local batches):
  - conv stages: channel-on-partition "H-part" tiles (128, 2052) bf16,
    data in cols [2, 2050), zero padding columns for the shifted conv taps.
  - rwkv stages: (128, 2048) bf16 tiles, H on partitions, T on free dim.
  - The wkv scan runs on the vector engine via tensor_tensor_scan
    (alpha_t = 0.9*alpha_{t-1} + s_t), then wkv_t = alpha_t + 0.1*alpha_{t-1}.
  - LayerNorm over H (partition axis): sums via ones-vector matmuls,
    per-t scalars broadcast back over partitions via K=1 matmuls.

kernel(**inputs) is self-contained: numpy preprocessing + bass build +
run_bass_kernel_spmd on cores 0..7 + gather.
"""
import os
import numpy as np
import ml_dtypes

import concourse.bass as bass
import concourse.bacc as bacc
import concourse.tile as tile
from concourse import mybir
from concourse.bass_utils import run_bass_kernel_spmd

EPS = 1e-5
B, T, C = 32, 2048, 64
NB, C5, H, L, NBLK, NCLS = 5, 320, 128, 3, 2, 4
CP = 384
NCORE = 8
BL = B // NCORE
NCH = 4
CH = 512
TP = T + 4      # padded width for band conv input
TF = T + 4      # feat tiles width (data cols 2..2050); >= T+2 needed
NT16 = 16       # 128-col chunks for transpose

PERM = np.array([(o % 64) * 5 + (o // 64) for o in range(C5)], dtype=np.int64)

F32 = mybir.dt.float32
F32R = mybir.dt.float32r
BF16 = mybir.dt.bfloat16
AF = mybir.ActivationFunctionType
ALU = mybir.AluOpType
bf16np = ml_dtypes.bfloat16


# ---------------------------------------------------------------------------
# host-side weight preprocessing (numpy only)
# ---------------------------------------------------------------------------

def _prep_weights(inp):
    f32 = np.float32
    out = {}

    bw = np.asarray(inp['band_w'], f32)[PERM, 0, :]
    bb = np.asarray(inp['band_b'], f32)[PERM]
    bw_pad = np.zeros((CP, 5), f32); bw_pad[:C5] = bw
    bb_pad = np.zeros((CP,), f32); bb_pad[:C5] = bb
    band_coef = bw_pad.reshape(3, 128, 5)

    bw_raw = np.asarray(inp['band_w'], f32)[:, 0, :].reshape(C, NB, 5)
    denom = f32(1.0 / (NB * T))
    A = bw_raw.sum(axis=(1, 2)) * denom
    E0 = -(bw_raw[:, :, 3] + bw_raw[:, :, 4]).sum(1) * denom
    E1 = -(bw_raw[:, :, 4]).sum(1) * denom
    E2 = -(bw_raw[:, :, 0]).sum(1) * denom
    E3 = -(bw_raw[:, :, 0] + bw_raw[:, :, 1]).sum(1) * denom
    Bb = np.asarray(inp['band_b'], f32).reshape(C, NB).mean(1)

    attn_rhs = np.zeros((65, 64), f32)
    attn_rhs[:64] = np.asarray(inp['attn_w'], f32).T
    attn_rhs[64] = np.asarray(inp['attn_b'], f32)
    out['attn_rhs'] = attn_rhs

    res_lhsT = np.zeros((4, 3, 3, 3, 128, 128), f32)
    res_bias = np.zeros((4, CP), f32)
    ci = 0
    for blk in range(NBLK):
        for lyr in range(2):
            W = np.asarray(inp['res_w'], np.float32)[blk, lyr]
            g = np.asarray(inp['res_bn_g'], f32)[blk, lyr]
            b = np.asarray(inp['res_bn_b'], f32)[blk, lyr]
            m = np.asarray(inp['res_bn_m'], f32)[blk, lyr]
            v = np.asarray(inp['res_bn_v'], f32)[blk, lyr]
            inv = g / np.sqrt(v + EPS)
            Wf = W * inv[:, None, None]
            bf = b - m * inv
            Wp = Wf[PERM][:, PERM]
            Wpad = np.zeros((CP, CP, 3), f32); Wpad[:C5, :C5] = Wp
            bpad = np.zeros((CP,), f32); bpad[:C5] = bf[PERM]
            res_bias[ci] = bpad
            WT = Wpad.transpose(1, 0, 2)
            for k in range(3):
                for q in range(3):
                    for mm in range(3):
                        res_lhsT[ci, k, q, mm] = \
                            WT[q*128:(q+1)*128, mm*128:(mm+1)*128, k]
            ci += 1
    out['res_lhsT'] = np.ascontiguousarray(
        res_lhsT.transpose(4, 0, 1, 2, 3, 5)).astype(bf16np)

    pw = np.asarray(inp['proj_w'], f32)[:, PERM]
    pw_pad = np.zeros((H, CP), f32); pw_pad[:, :C5] = pw
    out['proj_lhsT'] = np.ascontiguousarray(
        pw_pad.T.reshape(3, 128, H).transpose(1, 0, 2)).astype(bf16np)

    rwkv_lhsT = np.zeros((L, 4, H, H), f32)
    for l in range(L):
        rwkv_lhsT[l, 0] = np.asarray(inp['wk'], f32)[l].T
        rwkv_lhsT[l, 1] = np.asarray(inp['wv'], f32)[l].T
        rwkv_lhsT[l, 2] = np.asarray(inp['wr'], f32)[l].T
        rwkv_lhsT[l, 3] = np.asarray(inp['wo'], f32)[l].T
    out['rwkv_lhsT'] = np.ascontiguousarray(
        rwkv_lhsT.transpose(2, 0, 1, 3)).astype(bf16np)

    w1 = np.asarray(inp['cls_w1'], f32)
    out['cls1_lhsT'] = np.ascontiguousarray(w1.T.reshape(H, 2, 128))
    w2 = np.asarray(inp['cls_w2'], f32)
    out['cls2_lhsT'] = np.ascontiguousarray(
        w2.T.reshape(2, 128, NCLS).transpose(1, 0, 2))

    cols = {}
    def vec(name, v):
        cols[name] = np.asarray(v, f32)
    def pad128(v):
        o = np.zeros(128, f32); o[:len(v)] = v; return o

    for i in range(3):
        for k in range(5):
            vec(f'band_c{i}_{k}', band_coef[i, :, k])
    vec('A', pad128(A)); vec('E0', pad128(E0)); vec('E1', pad128(E1))
    vec('E2', pad128(E2)); vec('E3', pad128(E3)); vec('Bb', pad128(Bb))
    for c4 in range(4):
        for mm in range(3):
            vec(f'res_b{c4}_{mm}', res_bias[c4, mm*128:(mm+1)*128])
    vec('proj_b', np.asarray(inp['proj_b'], f32))
    for l in range(L):
        for w, nm in enumerate(['tmk', 'tmv', 'tmr']):
            tm = np.asarray(inp[nm], f32)[l]
            vec(f'tm{l}_{w}', tm)
            vec(f'tm1_{l}_{w}', (1.0 - tm) / T)
        vec(f'ln1g_{l}', np.asarray(inp['ln1g'], f32)[l])
        vec(f'ln1b_{l}', np.asarray(inp['ln1b'], f32)[l])
        vec(f'ln2g_{l}', np.asarray(inp['ln2g'], f32)[l])
        vec(f'ln2b_{l}', np.asarray(inp['ln2b'], f32)[l])
    for i in range(3):
        vec(f'band_bias_{i}', bb_pad.reshape(3, 128)[i])
    vec('cls_b1a', np.asarray(inp['cls_b1'], f32)[:128])
    vec('cls_b1b', np.asarray(inp['cls_b1'], f32)[128:])
    vec('cls_b2', pad128(np.asarray(inp['cls_b2'], f32)))
    vec('eps', np.full(128, EPS, f32))

    names = list(cols.keys())
    out['cvec'] = np.ascontiguousarray(np.stack([cols[n] for n in names], 1))
    out['cvec_idx'] = {n: i for i, n in enumerate(names)}
    out['identity'] = np.eye(128, dtype=f32)
    return out


# ---------------------------------------------------------------------------
# bass kernel builder
# ---------------------------------------------------------------------------

def _build_nc(nv, prep, dbg_keys=()):
    """nv = number of cvec columns. All weights are baked into the NEFF via
    inline_tensor (loaded to HBM once at model load); only x is a per-exec
    input."""
    nc = bacc.Bacc(None, target_bir_lowering=False)

    d_x = nc.dram_tensor('x', [BL, 64, T], BF16, kind='ExternalInput')
    d_cvec = nc.inline_tensor(prep['cvec'], name='cvec')
    d_attn = nc.inline_tensor(prep['attn_rhs'].astype(np.float32), name='attn_rhs')
    d_res = nc.inline_tensor(prep['res_lhsT'], name='res_lhsT')
    d_proj = nc.inline_tensor(prep['proj_lhsT'], name='proj_lhsT')
    d_rwkv = nc.inline_tensor(prep['rwkv_lhsT'], name='rwkv_lhsT')
    d_cls1 = nc.inline_tensor(prep['cls1_lhsT'].astype(np.float32), name='cls1_lhsT')
    d_cls2 = nc.inline_tensor(prep['cls2_lhsT'].astype(np.float32), name='cls2_lhsT')
    d_out = nc.dram_tensor('out', [NCLS, BL], F32, kind='ExternalOutput')

    with tile.TileContext(nc) as tc:
        _emit(nc, tc, d_x, d_cvec, d_attn, d_res, d_proj, d_rwkv,
              d_cls1, d_cls2, d_out, nv, dbg_keys)
    nc.finalize()
    return nc


def _emit(nc, tc, d_x, d_cvec, d_attn, d_res, d_proj, d_rwkv,
          d_cls1, d_cls2, d_out, nv, dbg_keys=()):
    from contextlib import ExitStack

    def cap(key, ap):
        if key in dbg_keys:
            dt = nc.dram_tensor(f'dbg_{key}', list(ap.shape),
                                ap.dtype, kind='ExternalOutput')
            nc.gpsimd.dma_start(out=dt[...], in_=ap)

    ctx = ExitStack()
    with ctx:
        consts = ctx.enter_context(tc.tile_pool(name='consts', bufs=1))
        big = ctx.enter_context(tc.tile_pool(name='big', bufs=28))
        stats = ctx.enter_context(tc.tile_pool(name='stats', bufs=1))
        small = ctx.enter_context(tc.tile_pool(name='small', bufs=1))
        xload = ctx.enter_context(tc.tile_pool(name='xload', bufs=4))
        psum = ctx.enter_context(tc.tile_pool(name='psum', bufs=8, space='PSUM'))
        psum_s = psum

        def bigt(name):
            return big.tile([128, TF], BF16, tag='big', name=name)

        # ---------------- constants -----------------
        cvec = consts.tile([128, nv], F32)
        nc.gpsimd.dma_start(out=cvec, in_=d_cvec[:, :])
        CV = {}

        def colap(name):
            return cvec[:, CV[name]:CV[name]+1]

        idx = 0
        def reg(name):
            nonlocal idx
            CV[name] = idx; idx += 1
        for i in range(3):
            for k in range(5):
                reg(f'band_c{i}_{k}')
        for n in ['A', 'E0', 'E1', 'E2', 'E3', 'Bb']:
            reg(n)
        for c4 in range(4):
            for mm in range(3):
                reg(f'res_b{c4}_{mm}')
        reg('proj_b')
        for l in range(L):
            for w in range(3):
                reg(f'tm{l}_{w}')
                reg(f'tm1_{l}_{w}')
            for n in [f'ln1g_{l}', f'ln1b_{l}', f'ln2g_{l}', f'ln2b_{l}']:
                reg(n)
        for i in range(3):
            reg(f'band_bias_{i}')
        for n in ['cls_b1a', 'cls_b1b', 'cls_b2', 'eps']:
            reg(n)
        assert idx == nv, (idx, nv)

        ones_l = consts.tile([128, 1], BF16)
        nc.vector.memset(ones_l, 1.0)
        decay = consts.tile([128, T], F32)
        nc.vector.memset(decay, 0.9)
        # f32r tiles cannot be memset directly (invalid ISA); synthesize via
        # ACT: out = Copy(in*0 + 1)
        ones_lf = consts.tile([128, 128], F32R)
        nc.scalar.activation(out=ones_lf, in_=decay[:, 0:128], func=AF.Copy,
                             bias=1.0, scale=0.0)

        attn_rhs = consts.tile([65, 64], F32R)
        nc.gpsimd.dma_start(out=attn_rhs, in_=d_attn[:, :])

        w_res = consts.tile([128, 4, 3, 3, 3, 128], BF16)
        nc.gpsimd.dma_start(out=w_res, in_=d_res[...])
        w_proj = consts.tile([128, 3, H], BF16)
        nc.gpsimd.dma_start(out=w_proj, in_=d_proj[...])
        w_rwkv = consts.tile([128, L, 4, H], BF16)
        nc.gpsimd.dma_start(out=w_rwkv, in_=d_rwkv[...])
        w_cls1 = consts.tile([128, 2, 128], F32R)
        nc.gpsimd.dma_start(out=w_cls1, in_=d_cls1[...])
        w_cls2 = consts.tile([128, 2, NCLS], F32R)
        nc.gpsimd.dma_start(out=w_cls2, in_=d_cls2[...])

        # ---------------- stage 1: load x, band conv ------------
        # x arrives as (BL, 64, T) bf16 (channels on partition). Duplicate
        # into rows [c; c] on-device via two DMAs and zero the 2 pad cols on
        # each side (band-conv input layout).
        xdup = [bigt(f'xdup{b}') for b in range(BL)]
        for b in range(BL):
            nc.vector.memset(xdup[b][:, 0:2], 0.0)
            nc.vector.memset(xdup[b][:, 2+T:4+T], 0.0)
            nc.sync.dma_start(out=xdup[b][0:64, 2:2+T], in_=d_x[b, :, :])
            nc.sync.dma_start(out=xdup[b][64:128, 2:2+T], in_=d_x[b, :, :])
        S_b = small.tile([64, BL], F32)
        for b in range(BL):
            nc.vector.tensor_reduce(
                out=S_b[:, b:b+1], in_=xdup[b][0:64, 2:2+T],
                axis=mybir.AxisListType.X, op=ALU.add)
        cap('xdup0', xdup[0][:, :])
        cap('S_b', S_b[:, :])

        F = [[bigt(f'F{b}_{i}') for i in range(3)] for b in range(BL)]
        O = [[bigt(f'O{b}_{i}') for i in range(3)] for b in range(BL)]
        for b in range(BL):
            for i in range(3):
                nc.vector.memset(F[b][i], 0.0)
                nc.vector.memset(O[b][i], 0.0)

        # band conv: F[b][i][:, 2:2+T] = sum_k coef_ik * xdup[b][:, k:k+T]
        for b in range(BL):
            for i in range(3):
                rows = 128 if i < 2 else 64
                dst = F[b][i][0:rows, 2:2+T]
                for k in range(5):
                    src = xdup[b][0:rows, k:k+T]
                    cf = colap(f'band_c{i}_{k}')[0:rows]
                    if k == 0:
                        nc.vector.tensor_scalar(
                            out=dst, in0=src, scalar1=cf, scalar2=None,
                            op0=ALU.mult)
                    else:
                        nc.vector.scalar_tensor_tensor(
                            out=dst, in0=src, scalar=cf, in1=dst,
                            op0=ALU.mult, op1=ALU.add)

        for i in range(3):
            cap(f'F0{i}_band', F[0][i][:, :])
        # pooled (64, BL)
        pooledT = small.tile([65, BL], F32R)
        nc.scalar.activation(out=pooledT[64:65, :], in_=S_b[0:1, 0:BL],
                             func=AF.Copy, bias=1.0, scale=0.0)
        for b in range(BL):
            p = pooledT[0:64, b:b+1]
            nc.vector.tensor_scalar(
                out=p, in0=S_b[:, b:b+1], scalar1=colap('A')[0:64],
                scalar2=colap('Bb')[0:64], op0=ALU.mult, op1=ALU.add)
            for name, cc in [('E0', 2), ('E1', 3), ('E2', T), ('E3', T+1)]:
                nc.vector.scalar_tensor_tensor(
                    out=p, in0=xdup[b][0:64, cc:cc+1],
                    scalar=colap(name)[0:64], in1=p,
                    op0=ALU.mult, op1=ALU.add)

        # attention: softmax over the 64 channels, computed in transposed
        # form (64, BL) so no PE transpose is needed. Logits are O(1) so the
        # max-subtraction can be dropped.
        att_ps = psum_s.tile([64, BL], F32, tag='mm512', name='att_ps')
        nc.tensor.matmul(att_ps, attn_rhs, pooledT)
        attE = small.tile([64, BL], F32R)
        nc.scalar.activation(out=attE, in_=att_ps, func=AF.Exp)
        sum_ps = psum_s.tile([1, BL], F32, tag='mm512', name='sum_ps')
        nc.tensor.matmul(sum_ps, ones_lf[0:64, 0:1], attE)
        arec = small.tile([1, BL], F32R)
        with nc.allow_low_precision(reason='softmax denom in fp32r is fine'):
            nc.vector.reciprocal(out=arec, in_=sum_ps)
        bc_ps = psum_s.tile([64, BL], F32, tag='mm512', name='bc_ps')
        nc.tensor.matmul(bc_ps, ones_lf[0:1, 0:64], arec, tile_position=(0, 0))
        attT = small.tile([64, BL], F32)
        nc.vector.tensor_tensor(out=attT, in0=attE, in1=bc_ps, op=ALU.mult)
        cap('pooledT', pooledT[:, :])
        cap('attT', attT[:, :])
        avec = [[small.tile([128, 1], F32, tag='avec', bufs=12,
                            name=f'avec{b}_{i}') for i in range(3)]
                for b in range(BL)]
        bxa = [[small.tile([128, 1], F32, tag='bxa', bufs=12,
                           name=f'bxa{b}_{i}') for i in range(3)]
               for b in range(BL)]
        for b in range(BL):
            for i in range(3):
                nc.vector.memset(avec[b][i], 0.0)
                nc.gpsimd.dma_start(out=avec[b][i][0:64, :], in_=attT[:, b:b+1])
                if i < 2:
                    nc.gpsimd.dma_start(out=avec[b][i][64:128, :],
                                      in_=attT[:, b:b+1])
                nc.vector.tensor_tensor(
                    out=bxa[b][i], in0=avec[b][i],
                    in1=colap(f'band_bias_{i}'), op=ALU.mult)
        for b in range(BL):
            for i in range(3):
                nc.vector.tensor_scalar(
                    out=F[b][i][:, 2:2+T], in0=F[b][i][:, 2:2+T],
                    scalar1=avec[b][i], scalar2=bxa[b][i],
                    op0=ALU.mult, op1=ALU.add)

        # ---------------- stage 2: resnet ---------------------------------
        def conv(c4, IN, OUT, residual):
            groups = [(b, n) for b in range(BL) for n in range(NCH)]
            for m in range(3):
                for gi in range(0, 16, 8):
                    gs = groups[gi:gi+8]
                    pts = [psum.tile([128, CH], F32, tag='mm512',
                                     name=f'cvp{c4}_{m}_{gi}_{g}')
                           for g in range(len(gs))]
                    first = True
                    for k in range(3):
                        for q in range(3):
                            lhsT = w_res[:, c4, k, q, m, :]
                            for (b, n), pt in zip(gs, pts):
                                rhs = IN[b][q][:, 1 + CH*n + k: 1 + CH*n + k + CH]
                                nc.tensor.matmul(
                                    pt, lhsT, rhs, start=first,
                                    stop=(k == 2 and q == 2))
                            first = False
                    bias = colap(f'res_b{c4}_{m}')
                    for (b, n), pt in zip(gs, pts):
                        dst = OUT[b][m][:, 2 + CH*n: 2 + CH*(n+1)]
                        if not residual:
                            nc.scalar.activation(out=dst, in_=pt, func=AF.Relu,
                                                 bias=bias, scale=1.0)
                        else:
                            tmp = xload.tile([128, CH], BF16, tag='cvt',
                                             name=f'cvt{c4}_{m}_{gi}_{b}_{n}')
                            nc.vector.scalar_tensor_tensor(
                                out=tmp, in0=pt, scalar=bias, in1=dst,
                                op0=ALU.add, op1=ALU.add)
                            nc.vector.tensor_scalar(
                                out=dst, in0=tmp, scalar1=0.0, scalar2=None,
                                op0=ALU.max)

        cap('F00_scaled', F[0][0][:, :])
        if 'noconv' not in os.environ.get('KABL', ''):
            conv(0, F, O, residual=False)
            cap('O00_c1', O[0][0][:, :])
            conv(1, O, F, residual=True)
            cap('F00_b1', F[0][0][:, :])
            conv(2, F, O, residual=False)
            conv(3, O, F, residual=True)
        cap('F00_res', F[0][0][:, :])

        # ---------------- stage 3: proj -----------------------------------
        h = [bigt(f'h{b}') for b in range(BL)]
        sums = [small.tile([128, 1], F32, tag='hsum', bufs=8,
                           name=f'hsum{b}') for b in range(BL)]
        for b in range(BL):
            for n in range(NCH):
                pt = psum.tile([128, CH], F32, tag='mm512', name=f'pjp{b}_{n}')
                for q in range(3):
                    nc.tensor.matmul(pt, w_proj[:, q, :],
                                     F[b][q][:, 2 + CH*n: 2 + CH*(n+1)],
                                     start=(q == 0), stop=(q == 2))
                nc.scalar.activation(out=h[b][:, CH*n:CH*(n+1)], in_=pt,
                                     func=AF.Identity, bias=colap('proj_b'),
                                     scale=1.0)
            nc.vector.tensor_reduce(out=sums[b], in_=h[b][:, 0:T],
                                    axis=mybir.AxisListType.X, op=ALU.add)
        cap('h0', h[0][:, 0:T])

        # ---------------- stage 4: rwkv layers -----------------------------
        nlayers = 0 if 'norwkv' in os.environ.get('KABL', '') else L
        for l in range(nlayers):
            h, sums = _rwkv_layer(nc, big, bigt, small, xload, psum, psum_s,
                                  stats, colap, w_rwkv, ones_l, ones_lf,
                                  decay, h, sums, l, cap)
            cap(f'hn{l}_0', h[0][:, 0:T])

        # ---------------- stage 5: head ------------------------------------
        pooledHf = small.tile([128, BL], F32R)
        for b in range(BL):
            nc.vector.tensor_scalar(out=pooledHf[:, b:b+1], in0=sums[b],
                                    scalar1=1.0 / T, scalar2=None, op0=ALU.mult)
        hidT = small.tile([128, 2, BL], F32R)
        for mt in range(2):
            pt = psum_s.tile([128, BL], F32, tag='mm512', name=f'clsp{mt}')
            nc.tensor.matmul(pt, w_cls1[:, mt, :],
                             pooledHf)
            nc.scalar.activation(out=hidT[:, mt, :], in_=pt, func=AF.Relu,
                                 bias=colap('cls_b1a' if mt == 0 else 'cls_b1b'),
                                 scale=1.0)
        out_ps = psum_s.tile([NCLS, BL], F32, tag='mm512', name='out_ps')
        for kt in range(2):
            nc.tensor.matmul(out_ps, w_cls2[:, kt, :],
                             hidT[:, kt, :],
                             start=(kt == 0), stop=(kt == 1))
        cap('pooledHf', pooledHf[:, :])
        cap('hidT', hidT[:, :, :])
        out_sb = small.tile([NCLS, BL], F32)
        nc.scalar.activation(out=out_sb, in_=out_ps, func=AF.Identity,
                             bias=colap('cls_b2')[0:NCLS], scale=1.0)
        nc.gpsimd.dma_start(out=d_out[:, :], in_=out_sb)


def _rwkv_layer(nc, big, bigt, small, xload, psum, psum_s, stats, colap,
                w_rwkv, ones_l, ones_lf, decay, h, sums, l, cap=lambda *a: None):
    # xk/xv/xr
    xs = [[bigt(f'xs{l}_{b}_{w}') for w in range(3)] for b in range(BL)]
    for b in range(BL):
        for w in range(3):
            tmv1 = small.tile([128, 1], F32, tag='tmv1', bufs=4,
                              name=f'tmv1_{l}_{b}_{w}')
            nc.vector.tensor_tensor(out=tmv1, in0=sums[b],
                                    in1=colap(f'tm1_{l}_{w}'), op=ALU.mult)
            nc.vector.tensor_scalar(
                out=xs[b][w][:, 0:T], in0=h[b][:, 0:T],
                scalar1=colap(f'tm{l}_{w}'),
                scalar2=tmv1, op0=ALU.mult, op1=ALU.add)
    sk = [bigt(f'sk{l}_{b}') for b in range(BL)]
    vv = [bigt(f'vv{l}_{b}') for b in range(BL)]
    rr = [bigt(f'rr{l}_{b}') for b in range(BL)]
    for b in range(BL):
        for w, (dst, fn) in enumerate([(sk[b], AF.Sigmoid), (vv[b], AF.Relu),
                                       (rr[b], AF.Sigmoid)]):
            for n in range(NCH):
                pt = psum.tile([128, CH], F32, tag='mm512',
                               name=f'kvr{l}_{b}_{w}_{n}')
                nc.tensor.matmul(pt, w_rwkv[:, l, w, :],
                                 xs[b][w][:, CH*n:CH*(n+1)])
                nc.scalar.activation(out=dst[:, CH*n:CH*(n+1)], in_=pt, func=fn)
    ss = [bigt(f'ss{l}_{b}') for b in range(BL)]
    alpha = [bigt(f'alpha{l}_{b}') for b in range(BL)]
    rwkv = [bigt(f'rwkv{l}_{b}') for b in range(BL)]
    for b in range(BL):
        nc.vector.scalar_tensor_tensor(
            out=ss[b][:, 0:T], in0=sk[b][:, 0:T], scalar=0.5,
            in1=vv[b][:, 0:T], op0=ALU.max, op1=ALU.mult)
        nc.vector.memset(alpha[b][:, 0:1], 0.0)
        import os as _os
        if 'noscan' in _os.environ.get('KABL', ''):
            nc.vector.tensor_copy(out=alpha[b][:, 1:T+1], in_=ss[b][:, 0:T])
        else:
            nc.vector.tensor_tensor_scan(
                out=alpha[b][:, 1:T+1], data0=decay, data1=ss[b][:, 0:T],
                initial=0.0, op0=ALU.mult, op1=ALU.add)
        nc.vector.scalar_tensor_tensor(
            out=ss[b][:, 0:T], in0=alpha[b][:, 0:T], scalar=0.1,
            in1=alpha[b][:, 1:T+1], op0=ALU.mult, op1=ALU.add)
        nc.vector.tensor_tensor(out=rwkv[b][:, 0:T], in0=rr[b][:, 0:T],
                                in1=ss[b][:, 0:T], op=ALU.mult)
    if l == 0:
        cap('xs00', xs[0][0][:, 0:T])
        cap('sk00', sk[0][:, 0:T])
        cap('vv00', vv[0][:, 0:T])
        cap('rr00', rr[0][:, 0:T])
        cap('alpha00', alpha[0][:, 0:T+1])
        cap('rwkv00', rwkv[0][:, 0:T])
    y = [bigt(f'y{l}_{b}') for b in range(BL)]
    for b in range(BL):
        for n in range(NCH):
            pt = psum.tile([128, CH], F32, tag='mm512', name=f'op{l}_{b}_{n}')
            nc.tensor.matmul(pt, w_rwkv[:, l, 3, :], rwkv[b][:, CH*n:CH*(n+1)])
            nc.vector.tensor_tensor(out=y[b][:, CH*n:CH*(n+1)], in0=pt,
                                    in1=h[b][:, CH*n:CH*(n+1)], op=ALU.add)

    if l == 0:
        cap('y00', y[0][:, 0:T])
    yn = [bigt(f'yn{l}_{b}') for b in range(BL)]
    ffp = [bigt(f'ffp{l}_{b}') for b in range(BL)]
    hn = [bigt(f'hn{l}_{b}') for b in range(BL)]
    nsums = [small.tile([128, 1], F32, tag='hsum', bufs=8,
                        name=f'nsums{l}_{b}') for b in range(BL)]
    _ln(nc, big, bigt, small, xload, psum, stats, colap, ones_l, ones_lf,
        y, yn, f'ln1g_{l}', f'ln1b_{l}', tagp=f'l{l}a')
    _ln(nc, big, bigt, small, xload, psum, stats, colap, ones_l, ones_lf,
        yn, ffp, f'ln2g_{l}', f'ln2b_{l}', tagp=f'l{l}b')
    if l == 0:
        cap('yn00', yn[0][:, 0:T])
        cap('ffp00', ffp[0][:, 0:T])
    for b in range(BL):
        nc.vector.scalar_tensor_tensor(
            out=hn[b][:, 0:T], in0=ffp[b][:, 0:T], scalar=0.0,
            in1=yn[b][:, 0:T], op0=ALU.max, op1=ALU.add, accum_out=nsums[b])
    return hn, nsums


def _ln(nc, big, bigt, small, xload, psum, stats, colap, ones_l, ones_lf,
        y, out, gname, bname, tagp):
    """LayerNorm over the partition axis for each (batch, t) column.
    Stats rows live at partition 32*b of (128, T) f32 tiles."""
    stat_y = stats.tile([128, T], F32R, tag='stat_y', name=f'sty_{tagp}')
    stat_q = stats.tile([128, T], F32R, tag='stat_q', name=f'stq_{tagp}')
    stat_v = stats.tile([128, T], F32, tag='stat_v', name=f'stv_{tagp}')
    ysq = [bigt(f'ysq{tagp}_{b}') for b in range(BL)]
    for b in range(BL):
        nc.scalar.activation(out=ysq[b][:, 0:T], in_=y[b][:, 0:T],
                             func=AF.Square)
    for n in range(NCH):
        p1 = psum.tile([128, CH], F32, tag='mm512', name=f'st1_{tagp}_{n}')
        p2 = psum.tile([128, CH], F32, tag='mm512', name=f'st2_{tagp}_{n}')
        for b in range(BL):
            nc.tensor.matmul(p1[32*b:32*b+1, :], ones_l,
                             y[b][:, CH*n:CH*(n+1)], tile_position=(0, 32*b))
            nc.tensor.matmul(p2[32*b:32*b+1, :], ones_l,
                             ysq[b][:, CH*n:CH*(n+1)], tile_position=(0, 32*b))
        nc.scalar.activation(out=stat_y[:, CH*n:CH*(n+1)], in_=p1, func=AF.Copy,
                             scale=1.0 / H)
        nc.scalar.activation(out=stat_q[:, CH*n:CH*(n+1)], in_=p2, func=AF.Copy,
                             scale=1.0 / H)
    sp = lambda t: t  # full-range ops; only rows 32*b are meaningful
    # var = e2 - mu^2 (into stat_q); sd = sqrt(var+eps) (stat_v);
    # inv = 1/sd (stat_q); negq = -mu*inv (stat_y)
    nc.vector.tensor_tensor(out=sp(stat_v), in0=sp(stat_y), in1=sp(stat_y),
                            op=ALU.mult)
    nc.vector.tensor_tensor(out=sp(stat_q), in0=sp(stat_q), in1=sp(stat_v),
                            op=ALU.subtract)
    nc.scalar.activation(out=sp(stat_v), in_=sp(stat_q), func=AF.Sqrt,
                         bias=colap('eps'), scale=1.0)
    with nc.allow_low_precision(reason='fp32r LN inv is plenty (FP22)'):
        nc.vector.reciprocal(out=sp(stat_q), in_=sp(stat_v))
    nc.vector.scalar_tensor_tensor(out=sp(stat_y), in0=sp(stat_y), scalar=-1.0,
                                   in1=sp(stat_q), op0=ALU.mult, op1=ALU.mult)
    inv, negq = stat_q, stat_y
    gv = colap(gname); bv = colap(bname)
    for b in range(BL):
        pb = bigt(f'bcP{tagp}_{b}')
        qb = bigt(f'bcQ{tagp}_{b}')
        for n in range(NCH):
            bp = psum.tile([128, CH], F32, tag='mm512', name=f'bp_{tagp}_{b}_{n}')
            bq = psum.tile([128, CH], F32, tag='mm512', name=f'bq_{tagp}_{b}_{n}')
            nc.tensor.matmul(bp, ones_lf[32*b:32*b+1, :],
                             inv[32*b:32*b+1, CH*n:CH*(n+1)],
                             tile_position=(32*b, 0))
            nc.tensor.matmul(bq, ones_lf[32*b:32*b+1, :],
                             negq[32*b:32*b+1, CH*n:CH*(n+1)],
                             tile_position=(32*b, 0))
            nc.scalar.activation(out=pb[:, CH*n:CH*(n+1)], in_=bp,
                                 func=AF.Identity, bias=0.0, scale=gv)
            nc.scalar.activation(out=qb[:, CH*n:CH*(n+1)], in_=bq,
                                 func=AF.Identity, bias=bv, scale=gv)
        t1 = bigt(f'lnt{tagp}_{b}')
        nc.vector.tensor_tensor(out=t1[:, 0:T], in0=y[b][:, 0:T],
                                in1=pb[:, 0:T], op=ALU.mult)
        nc.vector.tensor_tensor(out=out[b][:, 0:T], in0=t1[:, 0:T],
                                in1=qb[:, 0:T], op=ALU.add)


# ---------------------------------------------------------------------------
# entry point
# ---------------------------------------------------------------------------

_CACHE = {}


def kernel(**inputs):
    import hashlib
    wkey = hashlib.sha256()
    for k in sorted(inputs):
        if k != 'x':
            wkey.update(np.ascontiguousarray(np.asarray(inputs[k])).tobytes())
    wkey = wkey.hexdigest()
    if _CACHE.get('wkey') != wkey:
        prep = _prep_weights(inputs)
        nv = prep['cvec'].shape[1]
        _CACHE['nc'] = _build_nc(nv, prep)
        _CACHE['wkey'] = wkey
    nc = _CACHE['nc']

    x = np.asarray(inputs['x'], np.float32).astype(bf16np)
    xc = x.reshape(NCORE, BL, T, C).transpose(0, 1, 3, 2)   # (core, b, c, t)
    in_maps = [{'x': np.ascontiguousarray(xc[c])} for c in range(NCORE)]
    _CACHE['in_maps'] = in_maps
    res = run_bass_kernel_spmd(nc, in_maps, core_ids=list(range(NCORE)))
    outs = [res.results[c]['out'] for c in range(NCORE)]   # (NCLS, BL) each
    logits = np.concatenate([o.T for o in outs], axis=0)   # (B, NCLS)
    return logits.astype(np.float32)


def profile_exec_ns():
    """Re-run with NTFF tracing; returns max per-core exec time in ns."""
    if 'nc' not in _CACHE or 'in_maps' not in _CACHE:
        return None
    try:
        res = run_bass_kernel_spmd(_CACHE['nc'], _CACHE['in_maps'],
                                   core_ids=list(range(NCORE)), trace=True)
    except Exception as e:
        print("profile unavailable:", e)
        return None
    if res.instructions_and_trace is not None:
        print("trace:", res.instructions_and_trace[1])
    return res.exec_time_ns


def bench_exec(n=8):
    """Steady-state timing of the compiled SPMD executable (device-resident
    inputs, jit built once). Returns (min_s, avg_s) per call."""
    import time
    import jax
    from jax.sharding import Mesh, PartitionSpec
    from jax.experimental.shard_map import shard_map
    from concourse import bass2jax as b2j
    from concourse import mybir

    nc = _CACHE['nc']; in_maps = _CACHE['in_maps']
    b2j.install_neuronx_cc_hook()
    partition_name = nc.partition_id_tensor.name if nc.partition_id_tensor else None
    in_names, out_names, out_avals, zero_outs = [], [], [], []
    for alloc in nc.m.functions[0].allocations:
        if not isinstance(alloc, mybir.MemoryLocationSet):
            continue
        name = alloc.memorylocations[0].name
        if alloc.kind == 'ExternalInput':
            if name != partition_name:
                in_names.append(name)
        elif alloc.kind == 'ExternalOutput':
            sh = tuple(alloc.tensor_shape)
            dt = mybir.dt.np(alloc.dtype)
            out_avals.append(jax.core.ShapedArray(sh, dt))
            out_names.append(name)
            zero_outs.append(np.zeros(sh, dt))
    n_params = len(in_names)
    n_outs = len(out_avals)
    all_in_names = list(in_names) + list(out_names)
    if partition_name is not None:
        all_in_names.append(partition_name)

    def _body(*args):
        operands = list(args)
        if partition_name is not None:
            operands.append(b2j.partition_id_tensor())
        outs = b2j._bass_exec_p.bind(
            *operands, out_avals=tuple(out_avals), in_names=tuple(all_in_names),
            out_names=tuple(out_names), lowering_input_output_aliases=(),
            sim_require_finite=True, sim_require_nnan=True, nc=nc)
        return tuple(outs)

    devices = jax.devices()[:NCORE]
    mesh = Mesh(np.asarray(devices), ('core',))
    in_specs = (PartitionSpec('core'),) * (n_params + n_outs)
    out_specs = (PartitionSpec('core'),) * len(out_names)
    sharded = jax.jit(shard_map(_body, mesh=mesh, in_specs=in_specs,
                                out_specs=out_specs, check_rep=False),
                      keep_unused=True)
    concat_in = [np.concatenate([np.asarray(in_maps[c][nm])
                                 for c in range(NCORE)], axis=0)
                 for nm in in_names]
    concat_zeros = [np.zeros((NCORE * z.shape[0], *z.shape[1:]), z.dtype)
                    for z in zero_outs]
    args = [jax.device_put(a) for a in concat_in + concat_zeros]
    r = sharded(*args); jax.block_until_ready(r)   # warmup/compile
    def run_n(k):
        t0 = time.perf_counter()
        rs = [sharded(*args) for _ in range(k)]
        jax.block_until_ready(rs)
        return time.perf_counter() - t0
    run_n(2)
    t1 = min(run_n(1) for _ in range(3))
    tn = min(run_n(n) for _ in range(3))
    slope = (tn - t1) / (n - 1)
    return t1, slope

